# revision 10
# baseline (speedup 1.0000x reference)
"""Trainium2 Bass kernel for nn_AttentionSimple (sparse_attention, 8 cores).

Reference (per batch row b):
    e      = embeddings[k[b]]              # [S, E] gather
    scores = q[b] . e[s]                   # [S]
    attn   = softmax(scores); ctx = sum_s attn[s] * e[s]
    out    = ctx @ W.T + b                 # [B, 2]

Algorithm: count-weighted vocab-space softmax — no per-token gathers.
Scores depend on s only through v = k[b, s], so group softmax terms by
vocabulary id:
    c[b, v]  = |{s : k[b, s] = v}|         (histogram of k — a sufficient
                                            statistic, built on the host
                                            during input sharding)
    l[b, v]  = q[b] . embeddings[v]        (dense PE matmul)
    A        = c * exp(l)                  (no max-subtraction needed: the
                                            final ratio is shift-invariant
                                            and |l| < ~50 keeps exp in f32)
    out[b]   = (sum_v A[b,v] * EW[v]) / (sum_v A[b,v])
    with EW  = embeddings @ W.T + b        (parameter prepacking, host)

Sharding: the padded vocabulary (53248 = 416 chunks of 128) is split
across the 8 cores (52 chunks each); every core handles all 128 batch
rows. The embedding table is therefore read once in total. Each core
returns partial numerators/denominators; the host unshard step sums the
8 partials and divides (flash-style partial-softmax merge).

Per-core device pipeline (all engines ~full 128-partition utilization):
  - embT2: two vocab chunks stacked on the matmul contraction dim
    (embedding rows 0:50 -> even chunk, 64:114 -> odd chunk), so one
    f32r matmul per chunk-pair computes 256 logit columns:
        ps[128, 512] = embT2_pair.T @ [qT | qT]   (single PSUM bank)
  - ACT: A = exp(ps) fused PSUM->SBUF per quad (2 pairs)
  - DVE: A *= counts (uint8 transport, exact; asserted <= 255) in place
  - PE:  acc[9, 512] += st9_quad.T @ A  (f32r, N=512, single pass),
    st9 = [EW_chunk0..EW_chunk3 | ones]; 13 accumulating matmuls.
  - f32r (tf32-like rounding on the matmul inputs) halves PE passes vs
    f32 LOW/HIGH; measured end-to-end error vs the f32 reference: 6e-4
    relative to absmax (f32-everywhere variant measures 1.6e-6 but runs
    ~45% slower).
  - 3 warm-up matmuls on a zeroed tile run while the first DMAs land so
    the PE clock ramp (p-state) finishes before real work arrives.
  - All bulk DMAs ride one ordered Sync queue in exact consumption order
    (avoids SDMA round-robin starving the first tiles); tiny qw/st DMAs
    go on the Scalar queue.
"""

import numpy as np

BATCH, SEQ, EMB, VOCAB, OUT = 128, 8192, 50, 50000, 2
N_CORES = 8
CSH = 52                         # vocab chunks per core
NCHUNK = CSH * N_CORES           # 416
VPAD = NCHUNK * 128              # 53248
VSH = CSH * 128                  # 6656
NPAIR = CSH // 2                 # 26
NQUAD = NPAIR // 2               # 13
EPAD = 64
NQW = 2 * BATCH                  # 256 moving columns of mm1
GROUPS = [2, 2, 3, 3, 3]         # quads per DMA group

_CACHE = {}


def _build_nc():
    from contextlib import ExitStack

    import concourse.mybir as mybir
    import concourse.tile as tile
    from concourse import bacc

    f32 = mybir.dt.float32
    f32r = mybir.dt.float32r
    bf16 = mybir.dt.bfloat16
    nc = bacc.Bacc("TRN2", target_bir_lowering=False, debug=False,
                   num_devices=N_CORES)

    embT2_d = nc.dram_tensor("embT2", [128, NPAIR * 128], f32r,
                             kind="ExternalInput")
    qw_d = nc.dram_tensor("qw", [128, NQW], f32r, kind="ExternalInput")
    st_d = nc.dram_tensor("st", [128, NQUAD * 9], f32r,
                          kind="ExternalInput")
    ct_d = nc.dram_tensor("ct", [128, CSH * BATCH], mybir.dt.uint8,
                          kind="ExternalInput")
    o_d = nc.dram_tensor("o", [9, 4 * BATCH], f32, kind="ExternalOutput")

    with tile.TileContext(nc) as tc, ExitStack() as ctx:
        const_p = ctx.enter_context(tc.tile_pool(name="const", bufs=1))
        emb_p = ctx.enter_context(tc.tile_pool(name="embt", bufs=3))
        ct_p = ctx.enter_context(tc.tile_pool(name="ctp", bufs=3))
        le_p = ctx.enter_context(tc.tile_pool(name="le", bufs=6))
        ps_p = ctx.enter_context(tc.tile_pool(name="ps", bufs=6, space="PSUM"))
        acc_p = ctx.enter_context(tc.tile_pool(name="acc", bufs=1,
                                               space="PSUM"))
        wps_p = ctx.enter_context(tc.tile_pool(name="wps", bufs=1,
                                               space="PSUM"))
        fin_p = ctx.enter_context(tc.tile_pool(name="fin", bufs=1))

        # PE warm-up: matmuls on a zeroed tile, running while the first
        # input DMAs are in flight, so the PE p-state ramps early.
        wtile = const_p.tile([128, 512], f32r)
        nc.vector.memset(wtile[:].bitcast(f32), 0.0)
        wps = wps_p.tile([128, 512], f32)
        for _ in range(3):
            nc.tensor.matmul(wps[:], lhsT=wtile[:, 0:128], rhs=wtile[:],
                             start=True, stop=True)

        qw_sb = const_p.tile([128, NQW], f32r)
        nc.scalar.dma_start(qw_sb[:], qw_d.ap())
        st_sb = const_p.tile([128, NQUAD * 9], f32r)
        nc.scalar.dma_start(st_sb[:], st_d.ap())
        acc = acc_p.tile([9, 4 * BATCH], f32)

        quad0 = 0
        for gsz in GROUPS:
            et = emb_p.tile([128, 3 * 256], f32r, tag="et")
            nc.sync.dma_start(
                et[:, 0:gsz * 256],
                embT2_d.ap()[:, quad0 * 256:(quad0 + gsz) * 256])
            ctt = ct_p.tile([128, 3 * 512], mybir.dt.uint8, tag="ct")
            nc.sync.dma_start(
                ctt[:, 0:gsz * 512],
                ct_d.ap()[:, quad0 * 512:(quad0 + gsz) * 512])

            for lq in range(gsz):
                quad = quad0 + lq
                ps = ps_p.tile([128, 512], f32)
                for h in range(2):            # the quad's two pairs
                    nc.tensor.matmul(
                        ps[:, h * 256:(h + 1) * 256],
                        lhsT=et[:, lq * 256 + h * 128:lq * 256 + h * 128 + 128],
                        rhs=qw_sb[:],
                        start=True, stop=True,
                    )
                le = le_p.tile([128, 512], f32r)
                nc.scalar.activation(le[:], ps[:],
                                     mybir.ActivationFunctionType.Exp)
                nc.vector.tensor_mul(
                    le[:], le[:], ctt[:, lq * 512:(lq + 1) * 512])
                nc.tensor.matmul(
                    acc[:],
                    lhsT=st_sb[:, quad * 9:(quad + 1) * 9],
                    rhs=le[:],
                    start=(quad == 0), stop=(quad == NQUAD - 1),
                    skip_group_check=True,
                )
            quad0 += gsz

        osb = fin_p.tile([9, 4 * BATCH], f32)
        nc.vector.tensor_copy(osb[:], acc[:])
        nc.sync.dma_start(o_d.ap(), osb[:])

    nc.finalize()
    return nc


def _prep_inputs(q, k, embeddings, W, b):
    import ml_dtypes

    q = np.ascontiguousarray(q, dtype=np.float32)
    emb = np.ascontiguousarray(embeddings, dtype=np.float32)
    W = np.ascontiguousarray(W, dtype=np.float32)
    b = np.ascontiguousarray(b, dtype=np.float32)
    k = np.asarray(k)

    embT = np.zeros((EMB, VPAD), np.float32)
    embT[:, :VOCAB] = emb.T

    # mm1 moving operand: block-diagonal [qT | 0; 0 | qT]
    qw = np.zeros((128, NQW), np.float32)
    qw[:EMB, 0:BATCH] = q.T
    qw[EPAD:EPAD + EMB, BATCH:2 * BATCH] = q.T

    # weight prepacking: EW = emb @ W.T + b (function of parameters only)
    EWp = np.zeros((VPAD, OUT), np.float32)
    EWp[:VOCAB] = emb @ W.T + b[None, :]

    flat = (np.arange(BATCH, dtype=np.int64)[:, None] * VPAD
            + k.astype(np.int64)).ravel()
    C = np.bincount(flat, minlength=BATCH * VPAD).reshape(BATCH, VPAD)
    assert C.max() <= 255, "count histogram overflows uint8 transport"
    C = C.astype(np.float32)

    in_maps = []
    for core in range(N_CORES):
        v0 = core * VSH
        blocks = embT[:, v0:v0 + VSH].reshape(EMB, CSH, 128)
        e2 = np.zeros((128, NPAIR, 128), np.float32)
        e2[:EMB] = blocks[:, 0::2, :]
        e2[EPAD:EPAD + EMB] = blocks[:, 1::2, :]
        e2 = np.ascontiguousarray(e2.reshape(128, NPAIR * 128))

        # st9 per quad: cols 2j+o = EW[chunk j, o] laid per partition; col 8 = 1
        ew_blocks = EWp[v0:v0 + VSH].reshape(CSH, 128, OUT)  # [52, 128, 2]
        st = np.zeros((128, NQUAD, 9), np.float32)
        for j in range(4):
            st[:, :, 2 * j:2 * j + 2] = (
                ew_blocks.reshape(NQUAD, 4, 128, OUT)[:, j]
                .transpose(1, 0, 2))
        st[:, :, 8] = 1.0
        st = np.ascontiguousarray(st.reshape(128, NQUAD * 9))

        ct = np.ascontiguousarray(
            C[:, v0:v0 + VSH].reshape(BATCH, CSH, 128)
            .transpose(2, 1, 0).reshape(128, CSH * BATCH)
            .astype(np.uint8))
        in_maps.append({"embT2": e2, "qw": qw, "st": st, "ct": ct})
    return in_maps


def _run_device(in_maps, **kwargs):
    from concourse.bass_utils import run_bass_kernel_spmd

    if "nc" not in _CACHE:
        _CACHE["nc"] = _build_nc()
    return run_bass_kernel_spmd(_CACHE["nc"], in_maps,
                                core_ids=list(range(N_CORES)), **kwargs)


def _unshard(res):
    P = np.zeros((9, 4 * BATCH), np.float64)
    for i in range(N_CORES):
        P += res.results[i]["o"].astype(np.float64)
    numer = np.zeros((OUT, BATCH), np.float64)
    denom = np.zeros(BATCH, np.float64)
    for j in range(4):
        numer += P[2 * j:2 * j + 2, j * BATCH:(j + 1) * BATCH]
        denom += P[8, j * BATCH:(j + 1) * BATCH]
    out = (numer / denom[None, :]).T
    return np.ascontiguousarray(out, dtype=np.float32)


def kernel(q, k, embeddings, W, b, **_unused):
    in_maps = _prep_inputs(q, k, embeddings, W, b)
    res = _run_device(in_maps)
    return _unshard(res)
